# revision 1
# baseline (speedup 1.0000x reference)
"""EdgeEmbedding forward on 8 Trainium2 NeuronCores.

Computation (see reference):
    type_attr_sum[t] = sum_{j: attr_seg_ids[j]==t} attr_table[flat_attr_ids[j]]
    combined[t]      = edge_type_embedding[t] + type_attr_sum[t]        # [1000, 256]
    out[i]           = combined[data[i]]                                # [1M, 256]

Distribution:
  Stage 1 (segment sum): the 50K ragged attr references are sharded across
  the 8 cores by attr-table row range (core k owns attr_table rows
  [25000k, 25000(k+1)) and receives that table slice plus the (local-id, seg)
  pairs that fall in it). Each core gathers its rows with gpsimd.dma_gather,
  reduces them per segment with a one-hot PE matmul (bf16 hi/lo split, fp32
  PSUM accumulate -> ~2^-16 relative error), and the 8 partial [1000,256]
  sums are combined on-chip with an AllReduce; edge_type_embedding is added
  to form the combined table, replicated in every core's HBM.

  Stage 2 (edge gather): data is sharded across cores (125K edges each).
  Each core streams its indices through gpsimd.dma_gather from the 1MB
  combined table into edge-major SBUF tiles and writes them out with plain
  HWDGE DMAs.
"""
import os
import sys

sys.path.insert(0, "/opt/trn_rl_repo")

import numpy as np

import concourse.bass as bass
import concourse.bacc as bacc
import concourse.mybir as mybir
from concourse.tile import TileContext
from concourse.bass_utils import run_bass_kernel_spmd

# ---- problem constants (hardcoded per harness contract) ----
N = 1_000_000
D = 256
NSEG = 1000
NSEG_PAD = 1024          # 8 x 128 PE M-tiles
ATTR_NUM = 200_000
NCORES = 8
ATTR_PER_CORE = ATTR_NUM // NCORES      # 25_000 table rows per core
ASLOTS = 6656            # per-core attr work slots (max bucket 6397 for the fixed seed)
AKT = ASLOTS // 128      # 64 K-tiles
E = N // NCORES          # 125_000 edges per core
# stage-2 hybrid split: last N_E_TILES*128 edges go through the PE one-hot
# path; the first E_P through the gpsimd dma_gather path.
N_E_TILES = 408          # 128-edge tiles on the PE path
BATCH = 4                # PE tiles per one-hot batch (512 edges)
NBATCH = N_E_TILES // BATCH
E_E = N_E_TILES * 128    # 51_200
E_P = E - E_E            # 73_800
T2 = 4096                # edges per stage-2 gather call
C2 = T2 // 128           # 32
NFULL = E_P // T2        # 18 full calls
TAIL = E_P - NFULL * T2  # 72 valid tail edges
TAIL_PAD = ((TAIL + 127) // 128) * 128  # 128
CTAIL = TAIL_PAD // 128  # 1
EPAD = NFULL * T2 + TAIL_PAD            # 73_856
DIDX_COLS = EPAD // 16                  # 4616

_cached = {}


def _build_program():
    if "nc" in _cached:
        return _cached["nc"]
    nc = bacc.Bacc("TRN2", target_bir_lowering=False, debug=False, num_devices=NCORES)

    attr_shard = nc.dram_tensor("attr_shard", [ATTR_PER_CORE, D], mybir.dt.float32, kind="ExternalInput")
    edge_emb = nc.dram_tensor("edge_emb", [NSEG, D], mybir.dt.float32, kind="ExternalInput")
    aidx = nc.dram_tensor("aidx", [128, ASLOTS // 16], mybir.dt.int16, kind="ExternalInput")
    asegf = nc.dram_tensor("asegf", [128, AKT], mybir.dt.float32, kind="ExternalInput")
    didx = nc.dram_tensor("didx", [128, DIDX_COLS], mybir.dt.int16, kind="ExternalInput")
    dval = nc.dram_tensor("dval", [128, NBATCH * BATCH * 128], mybir.dt.float16, kind="ExternalInput")
    out = nc.dram_tensor("out", [E_P, D], mybir.dt.float32, kind="ExternalOutput")
    out2 = nc.dram_tensor("out2", [E_E, D], mybir.dt.float32, kind="ExternalOutput")

    ar_in = nc.dram_tensor("ar_in", [NSEG_PAD, D], mybir.dt.float32)
    ar_out = nc.dram_tensor("ar_out", [NSEG_PAD, D], mybir.dt.float32)
    combined = nc.dram_tensor("combined", [NSEG_PAD, D], mybir.dt.float32)

    f32 = mybir.dt.float32
    bf16 = mybir.dt.bfloat16

    with TileContext(nc) as tc:
        # ---------------- stage 1 ----------------
        with (
            tc.tile_pool(name="s1big", bufs=1) as s1big,
            tc.tile_pool(name="s1misc", bufs=1) as s1misc,
            tc.tile_pool(name="s1oh", bufs=2) as s1oh,
            tc.tile_pool(name="s1ps", bufs=1, space="PSUM") as s1ps,
            tc.tile_pool(name="s1out", bufs=2) as s1out,
        ):
            aidx_t = s1misc.tile([128, ASLOTS // 16], mybir.dt.int16)
            nc.sync.dma_start(out=aidx_t[:, :], in_=aidx.ap())
            asegf_t = s1misc.tile([128, AKT], f32)
            nc.sync.dma_start(out=asegf_t[:, :], in_=asegf.ap())

            gsem = nc.alloc_semaphore("swdge_gather_dma")
            atile = s1big.tile([128, AKT, D], f32, tag="atile")
            for h in range(2):
                nc.gpsimd.dma_gather(
                    out_ap=atile[:, h * (AKT // 2):(h + 1) * (AKT // 2), :],
                    in_ap=attr_shard.ap(),
                    idxs_ap=aidx_t[:, h * (ASLOTS // 32):(h + 1) * (ASLOTS // 32)],
                    num_idxs=ASLOTS // 2,
                    num_idxs_reg=ASLOTS // 2,
                    elem_size=D,
                    single_packet=False,
                )

            hi = s1big.tile([128, AKT, D], bf16, tag="hi")
            lo = s1big.tile([128, AKT, D], bf16, tag="lo")
            nc.vector.tensor_copy(hi[:, :, :], atile[:, :, :])
            # atile -= hi (in fp32), then lo = bf16(atile)
            nc.vector.tensor_tensor(atile[:, :, :], atile[:, :, :], hi[:, :, :], op=mybir.AluOpType.subtract)
            nc.vector.tensor_copy(lo[:, :, :], atile[:, :, :])

            iota_t = s1misc.tile([128, NSEG_PAD], f32)
            nc.gpsimd.iota(iota_t[:, :], [[1, NSEG_PAD]], channel_multiplier=0,
                           allow_small_or_imprecise_dtypes=True)

            ps = [s1ps.tile([128, D], f32, tag=f"ps{m}", name=f"ps{m}") for m in range(8)]
            for c in range(AKT):
                oh = s1oh.tile([128, NSEG_PAD], bf16, tag="oh")
                segcol = asegf_t[:, c:c + 1].broadcast_to((128, NSEG_PAD))
                nc.vector.tensor_tensor(oh[:, :], iota_t[:, :], segcol, op=mybir.AluOpType.is_equal)
                for m in range(8):
                    nc.tensor.matmul(
                        ps[m][:, :], oh[:, m * 128:(m + 1) * 128], hi[:, c, :],
                        start=(c == 0), stop=False,
                    )
                    nc.tensor.matmul(
                        ps[m][:, :], oh[:, m * 128:(m + 1) * 128], lo[:, c, :],
                        start=False, stop=(c == AKT - 1),
                    )
            for m in range(8):
                part = s1out.tile([128, D], f32, tag="part")
                nc.vector.tensor_copy(part[:, :], ps[m][:, :])
                nc.sync.dma_start(out=ar_in.ap()[m * 128:(m + 1) * 128, :], in_=part[:, :])

            nc.gpsimd.collective_compute(
                "AllReduce", mybir.AluOpType.add,
                replica_groups=[list(range(NCORES))],
                ins=[ar_in.ap().opt()],
                outs=[ar_out.ap().opt()],
            )

            for m in range(8):
                rows = 128 if m < 7 else NSEG - 896
                acc_t = s1out.tile([128, D], f32, tag="acc")
                emb_t = s1out.tile([128, D], f32, tag="emb")
                nc.sync.dma_start(out=acc_t[:, :], in_=ar_out.ap()[m * 128:(m + 1) * 128, :])
                nc.sync.dma_start(out=emb_t[:rows, :], in_=edge_emb.ap()[m * 128:m * 128 + rows, :])
                nc.vector.tensor_add(acc_t[:rows, :], acc_t[:rows, :], emb_t[:rows, :])
                nc.sync.dma_start(out=combined.ap()[m * 128:(m + 1) * 128, :], in_=acc_t[:, :])

        # ---------------- stage 2 ----------------
        fp16 = mybir.dt.float16
        with (
            tc.tile_pool(name="s2idx", bufs=1) as s2idx,
            tc.tile_pool(name="s2tab", bufs=1) as s2tab,
            tc.tile_pool(name="s2t", bufs=3) as s2t,
            tc.tile_pool(name="s2oh", bufs=2) as s2oh,
            tc.tile_pool(name="s2ps", bufs=2, space="PSUM") as s2ps,
            tc.tile_pool(name="s2st", bufs=3) as s2st,
        ):
            didx_t = s2idx.tile([128, DIDX_COLS], mybir.dt.int16)
            nc.scalar.dma_start(out=didx_t[:, :], in_=didx.ap())

            # fp16 hi/lo split of the combined table, K-major for the PE path
            ctmp = s2tab.tile([128, 8, D], f32)
            nc.sync.dma_start(out=ctmp[:, :, :],
                              in_=combined.ap().rearrange("(k p) d -> p k d", p=128))
            chi = s2tab.tile([128, 8, D], fp16)
            nc.vector.tensor_copy(chi[:, :, :], ctmp[:, :, :])

            iotak = s2tab.tile([128, 8], f32)
            nc.gpsimd.iota(iotak[:, :], [[128, 8]], channel_multiplier=1,
                           allow_small_or_imprecise_dtypes=True)
            iotak16 = s2tab.tile([128, 8], fp16)
            nc.vector.tensor_copy(iotak16[:, :], iotak[:, :])

            def pool_call(t):
                last = t == NFULL
                Tt = TAIL_PAD if last else T2
                Ct = CTAIL if last else C2
                tile2 = s2t.tile([128, C2, D], f32, tag="t2", name=f"t2_{t}")
                nc.gpsimd.dma_gather(
                    out_ap=tile2[:, :Ct, :],
                    in_ap=combined.ap(),
                    idxs_ap=didx_t[:, t * (T2 // 16): t * (T2 // 16) + Tt // 16],
                    num_idxs=Tt,
                    num_idxs_reg=Tt,
                    elem_size=D,
                    single_packet=False,
                )
                if not last:
                    dst = bass.AP(out, t * T2 * D, [[D, 128], [128 * D, C2], [1, D]])
                    nc.sync.dma_start(out=dst, in_=tile2[:, :, :])
                else:
                    base = NFULL * T2
                    full_chunks = TAIL // 128
                    rem = TAIL - full_chunks * 128
                    if full_chunks:
                        dst = bass.AP(out, base * D, [[D, 128], [128 * D, full_chunks], [1, D]])
                        nc.sync.dma_start(out=dst, in_=tile2[:, :full_chunks, :])
                    if rem:
                        dst2 = bass.AP(out, (base + full_chunks * 128) * D, [[D, rem], [1, D]])
                        nc.sync.dma_start(out=dst2, in_=tile2[:rem, full_chunks, :])

            def pe_batch(b):
                sv = s2oh.tile([128, BATCH * 128], fp16, tag="sv", name=f"sv{b}")
                nc.scalar.dma_start(out=sv[:, :],
                                    in_=dval.ap()[:, b * BATCH * 128:(b + 1) * BATCH * 128])
                oh = s2oh.tile([128, 8, BATCH * 128], fp16, tag="oh", name=f"oh{b}")
                for k in range(8):
                    nc.vector.tensor_tensor(
                        oh[:, k, :],
                        iotak16[:, k:k + 1].broadcast_to((128, BATCH * 128)),
                        sv[:, :],
                        op=mybir.AluOpType.is_equal,
                    )
                st = s2st.tile([128, BATCH, D], f32, tag="st", name=f"st{b}")
                for s_ in range(BATCH):
                    ps2 = s2ps.tile([128, D], f32, tag=f"ps2_{s_}", name=f"ps2_{b}_{s_}")
                    for k in range(8):
                        nc.tensor.matmul(ps2[:, :], oh[:, k, s_ * 128:(s_ + 1) * 128],
                                         chi[:, k, :], start=(k == 0), stop=(k == 7))
                    nc.scalar.copy(st[:, s_, :], ps2[:, :])
                dstp = bass.AP(out2, (b * BATCH * 128) * D,
                               [[D, 128], [128 * D, BATCH], [1, D]])
                nc.scalar.dma_start(out=dstp, in_=st[:, :, :])

            # interleave emission so the scheduler keeps all engines busy
            ncalls = NFULL + 1
            bi = 0
            for t in range(ncalls):
                pool_call(t)
                n_b = (NBATCH * (t + 1)) // ncalls
                while bi < n_b:
                    pe_batch(bi)
                    bi += 1
            while bi < NBATCH:
                pe_batch(bi)
                bi += 1

    nc.compile()
    _cached["nc"] = nc
    return nc


def _wrap16(arr):
    """Position j -> [j%16, j//16] layout expected by dma_gather idx tensors."""
    assert arr.shape[0] % 16 == 0
    return arr.reshape(arr.shape[0] // 16, 16).T


def _prep_in_maps(data, attr_table, edge_type_embedding, flat_attr_ids, attr_seg_ids):
    ids = np.asarray(flat_attr_ids).astype(np.int64)
    segs = np.asarray(attr_seg_ids).astype(np.int64)
    data = np.asarray(data).astype(np.int64)
    attr_table = np.ascontiguousarray(np.asarray(attr_table, dtype=np.float32))
    edge_emb = np.ascontiguousarray(np.asarray(edge_type_embedding, dtype=np.float32))

    in_maps = []
    for k in range(NCORES):
        lo_id, hi_id = k * ATTR_PER_CORE, (k + 1) * ATTR_PER_CORE
        sel = (ids >= lo_id) & (ids < hi_id)
        ids_k = ids[sel] - lo_id
        segs_k = segs[sel]
        nk = ids_k.shape[0]
        assert nk <= ASLOTS, f"attr bucket {k} overflow: {nk} > {ASLOTS}"
        aid = np.zeros(ASLOTS, np.int64)
        aid[:nk] = ids_k
        aseg = np.full(ASLOTS, -1.0, np.float32)
        aseg[:nk] = segs_k.astype(np.float32)

        aidx16 = np.tile(_wrap16(aid).astype(np.int16), (8, 1))          # [128, 512]
        asegf = np.ascontiguousarray(aseg.reshape(AKT, 128).T)           # [128, 64]

        shard = data[k * E:(k + 1) * E]
        shard_pool = shard[:E_P]
        shard_p = np.concatenate([shard_pool, np.zeros(EPAD - E_P, np.int64)])
        cols = []
        for t in range(NFULL + 1):
            Tt = TAIL_PAD if t == NFULL else T2
            sl = shard_p[t * T2: t * T2 + Tt]
            cols.append(_wrap16(sl))
        didx16 = np.tile(np.concatenate(cols, axis=1).astype(np.int16), (8, 1))
        dval16 = np.tile(shard[E_P:].astype(np.float16).reshape(1, -1), (128, 1))

        in_maps.append({
            "attr_shard": np.ascontiguousarray(attr_table[lo_id:hi_id]),
            "edge_emb": edge_emb,
            "aidx": np.ascontiguousarray(aidx16),
            "asegf": asegf,
            "didx": np.ascontiguousarray(didx16),
            "dval": np.ascontiguousarray(dval16),
        })
    return in_maps


def run(inputs, trace=False, trace_cores=None):
    nc = _build_program()
    in_maps = _prep_in_maps(**inputs)
    kwargs = {}
    if trace:
        kwargs = dict(trace=True)
        if trace_cores is not None:
            kwargs["trace_cores"] = trace_cores
    res = run_bass_kernel_spmd(nc, in_maps, core_ids=list(range(NCORES)), **kwargs)
    outp = np.concatenate(
        [np.concatenate([res.results[k]["out"], res.results[k]["out2"]], axis=0)
         for k in range(NCORES)], axis=0)
    return outp, res


def kernel(**inputs) -> np.ndarray:
    outp, _ = run(inputs, trace=False)
    return outp



# revision 2
# speedup vs baseline: 1.8652x; 1.8652x over previous
"""EdgeEmbedding forward on 8 Trainium2 NeuronCores.

Computation (see reference):
    type_attr_sum[t] = sum_{j: attr_seg_ids[j]==t} attr_table[flat_attr_ids[j]]
    combined[t]      = edge_type_embedding[t] + type_attr_sum[t]        # [1000, 256]
    out[i]           = combined[data[i]]                                # [1M, 256]

Distribution / algorithm:
  Stage 1 (segment sum): the 50K ragged attr references are sharded across
  the 8 cores by attr-table row range.  Within a core the references are
  bucketed by seg>>7 (8 buckets of 1024 padded slots); each bucket is
  gathered with gpsimd.dma_gather, cast bf16, and reduced with one-hot PE
  matmuls into that bucket's 128-seg PSUM tile.  Each 128-seg chunk is
  AllReduced separately (pipelined), edge_type_embedding is folded in on
  core 0 only (its input; zeros elsewhere), giving the combined table
  chunk-by-chunk.

  Stage 2 (edge gather): edges are sharded across cores (125K each) and,
  on the host, stably bucketed by type>>7 into 8 chunks padded to 16384
  rows.  The device holds the combined table in SBUF as fp16 and emits
  every output row with a one-hot matmul: for each 128-edge tile,
  oh[t_local, e] = (t_local == dval[e]) built on DVE (dval replicated
  across partitions by gpsimd.partition_broadcast), then
  PSUM[e, :] = oh.T @ chi_chunk.  ACT evacuates PSUM, 2MB HWDGE DMAs
  write HBM.  No HBM gather reads at all - the only bulk HBM traffic is
  the mandatory output write.  The host inverse-permutes rows on unshard.
"""
import os
import sys

sys.path.insert(0, "/opt/trn_rl_repo")

import numpy as np

import concourse.bass as bass
import concourse.bacc as bacc
import concourse.mybir as mybir
from concourse.tile import TileContext
from concourse.bass_utils import run_bass_kernel_spmd

# ---- problem constants (hardcoded per harness contract) ----
N = 1_000_000
D = 256
NSEG = 1000
NSEG_PAD = 1024
ATTR_NUM = 200_000
NCORES = 8
ATTR_PER_CORE = ATTR_NUM // NCORES      # 25_000 table rows per core
E = N // NCORES                         # 125_000 edges per core

NCHUNK = 8            # type chunks of 128
L = 16_384            # padded edge slots per chunk (6.5 sigma above 15625 mean)
SEG = 4096            # edges per one-hot build granule
SEGS_PER_CHUNK = L // SEG               # 4
TILES_PER_SEG = SEG // 128              # 32
GROUP = 16            # 128-edge tiles per output DMA (2 MB)
GROUPS_PER_SEG = TILES_PER_SEG // GROUP  # 2

S1_BUCKETS = 8        # seg chunks of 128
S1_SLOTS_PER_BUCKET = 1024              # 8.7 sigma above 781 mean
S1_KT_PER_BUCKET = S1_SLOTS_PER_BUCKET // 128   # 8
S1_SLOTS = S1_BUCKETS * S1_SLOTS_PER_BUCKET     # 8192

_cached = {}


def _build_program():
    if "nc" in _cached:
        return _cached["nc"]
    nc = bacc.Bacc("TRN2", target_bir_lowering=False, debug=False, num_devices=NCORES)

    f32 = mybir.dt.float32
    bf16 = mybir.dt.bfloat16
    fp16 = mybir.dt.float16

    attr_shard = nc.dram_tensor("attr_shard", [ATTR_PER_CORE, D], f32, kind="ExternalInput")
    edge_emb = nc.dram_tensor("edge_emb", [NSEG_PAD, D], f32, kind="ExternalInput")
    aidx = nc.dram_tensor("aidx", [128, S1_SLOTS // 16], mybir.dt.int16, kind="ExternalInput")
    asegf = nc.dram_tensor("asegf", [128, S1_BUCKETS * S1_KT_PER_BUCKET], bf16, kind="ExternalInput")
    iotaf = nc.dram_tensor("iotaf", [128, 128], bf16, kind="ExternalInput")
    iotap = nc.dram_tensor("iotap", [128, 1], fp16, kind="ExternalInput")
    dval = nc.dram_tensor("dval", [1, NCHUNK * L], fp16, kind="ExternalInput")
    out_dev = nc.dram_tensor("out_dev", [NCHUNK * L, D], f32, kind="ExternalOutput")

    ar_in = nc.dram_tensor("ar_in", [NSEG_PAD, D], f32)
    ar_out = nc.dram_tensor("ar_out", [NSEG_PAD, D], f32)

    with TileContext(nc) as tc:
        with (
            tc.tile_pool(name="misc", bufs=1) as misc,
            tc.tile_pool(name="s1a", bufs=2) as s1a,
            tc.tile_pool(name="s1oh", bufs=2) as s1oh,
            tc.tile_pool(name="s1ps", bufs=2, space="PSUM") as s1ps,
            tc.tile_pool(name="s1out", bufs=2) as s1out,
            tc.tile_pool(name="chip", bufs=2) as chip,
            tc.tile_pool(name="s2dv", bufs=3) as s2dv,
            tc.tile_pool(name="s2dvr", bufs=3) as s2dvr,
            tc.tile_pool(name="s2oh", bufs=3) as s2oh,
            tc.tile_pool(name="s2ps", bufs=6, space="PSUM") as s2ps,
            tc.tile_pool(name="s2st", bufs=3) as s2st,
        ):
            # ---- prologue: constants / index tables ----
            aidx_t = misc.tile([128, S1_SLOTS // 16], mybir.dt.int16)
            nc.sync.dma_start(out=aidx_t[:, :], in_=aidx.ap())
            asegf_t = misc.tile([128, S1_BUCKETS * S1_KT_PER_BUCKET], bf16)
            nc.sync.dma_start(out=asegf_t[:, :], in_=asegf.ap())
            iotaf_t = misc.tile([128, 128], bf16)
            nc.sync.dma_start(out=iotaf_t[:, :], in_=iotaf.ap())
            iotap_t = misc.tile([128, 1], fp16)
            nc.sync.dma_start(out=iotap_t[:, :], in_=iotap.ap())

            for b in range(NCHUNK):
                # ================= stage 1, seg bucket b =================
                atile = s1a.tile([128, S1_KT_PER_BUCKET, D], f32, tag="atile", name=f"atile{b}")
                nc.gpsimd.dma_gather(
                    out_ap=atile[:, :, :],
                    in_ap=attr_shard.ap(),
                    idxs_ap=aidx_t[:, b * (S1_SLOTS_PER_BUCKET // 16):(b + 1) * (S1_SLOTS_PER_BUCKET // 16)],
                    num_idxs=S1_SLOTS_PER_BUCKET,
                    num_idxs_reg=S1_SLOTS_PER_BUCKET,
                    elem_size=D,
                    single_packet=False,
                )
                abf = s1a.tile([128, S1_KT_PER_BUCKET, D], bf16, tag="abf", name=f"abf{b}")
                nc.vector.tensor_copy(abf[:, :, :], atile[:, :, :])

                ps1 = s1ps.tile([128, D], f32, tag="ps1", name=f"ps1_{b}")
                for c in range(S1_KT_PER_BUCKET):
                    kt = b * S1_KT_PER_BUCKET + c
                    oh1 = s1oh.tile([128, 128], bf16, tag="oh1", name=f"oh1_{b}_{c}")
                    nc.vector.tensor_tensor(
                        oh1[:, :],
                        asegf_t[:, kt:kt + 1].broadcast_to((128, 128)),
                        iotaf_t[:, :],
                        op=mybir.AluOpType.is_equal,
                    )
                    nc.tensor.matmul(
                        ps1[:, :], oh1[:, :], abf[:, c, :],
                        start=(c == 0), stop=(c == S1_KT_PER_BUCKET - 1),
                    )
                part = s1out.tile([128, D], f32, tag="part", name=f"part{b}")
                nc.scalar.copy(part[:, :], ps1[:, :])
                embt = s1out.tile([128, D], f32, tag="embt", name=f"embt{b}")
                nc.sync.dma_start(out=embt[:, :], in_=edge_emb.ap()[b * 128:(b + 1) * 128, :])
                nc.vector.tensor_add(part[:, :], part[:, :], embt[:, :])
                nc.sync.dma_start(out=ar_in.ap()[b * 128:(b + 1) * 128, :], in_=part[:, :])

                nc.gpsimd.collective_compute(
                    "AllReduce", mybir.AluOpType.add,
                    replica_groups=[list(range(NCORES))],
                    ins=[ar_in.ap()[b * 128:(b + 1) * 128, :].opt()],
                    outs=[ar_out.ap()[b * 128:(b + 1) * 128, :].opt()],
                )

                ctmp = s1out.tile([128, D], f32, tag="ctmp", name=f"ctmp{b}")
                nc.sync.dma_start(out=ctmp[:, :], in_=ar_out.ap()[b * 128:(b + 1) * 128, :])
                chi = chip.tile([128, D], fp16, tag="chi", name=f"chi{b}")
                nc.vector.tensor_copy(chi[:, :], ctmp[:, :])

                # ================= stage 2, type chunk b =================
                for s in range(SEGS_PER_CHUNK):
                    off = b * L + s * SEG
                    dv = s2dv.tile([1, SEG], fp16, tag="dv", name=f"dv{b}_{s}")
                    nc.sync.dma_start(out=dv[:, :], in_=dval.ap()[0:1, off:off + SEG])
                    dvr = s2dvr.tile([128, SEG], fp16, tag="dvr", name=f"dvr{b}_{s}")
                    nc.gpsimd.partition_broadcast(dvr[:, :], dv[:, :], 128)
                    oh2 = s2oh.tile([128, SEG], fp16, tag="oh2", name=f"oh2_{b}_{s}")
                    nc.vector.tensor_tensor(
                        oh2[:, :],
                        iotap_t[:, 0:1].broadcast_to((128, SEG)),
                        dvr[:, :],
                        op=mybir.AluOpType.is_equal,
                    )
                    for g in range(GROUPS_PER_SEG):
                        st = s2st.tile([128, GROUP, D], f32, tag="st", name=f"st{b}_{s}_{g}")
                        for j in range(GROUP // 2):
                            pp = s2ps.tile([128, 2, D], f32, tag="pp", name=f"pp{b}_{s}_{g}_{j}")
                            for h in range(2):
                                u = g * GROUP + j * 2 + h
                                nc.tensor.matmul(
                                    pp[:, h, :],
                                    oh2[:, u * 128:(u + 1) * 128],
                                    chi[:, :],
                                    start=True, stop=True,
                                )
                            nc.scalar.copy(st[:, j * 2:j * 2 + 2, :], pp[:, :, :])
                        row0 = off + g * GROUP * 128
                        dst = bass.AP(out_dev, row0 * D, [[D, 128], [128 * D, GROUP], [1, D]])
                        nc.sync.dma_start(out=dst, in_=st[:, :, :])

    nc.compile()
    _cached["nc"] = nc
    return nc


def _wrap16(arr):
    """Position j -> [j%16, j//16] layout expected by dma_gather idx tensors."""
    assert arr.shape[0] % 16 == 0
    return arr.reshape(arr.shape[0] // 16, 16).T


def _prep_in_maps(data, attr_table, edge_type_embedding, flat_attr_ids, attr_seg_ids):
    import ml_dtypes
    bf16 = ml_dtypes.bfloat16

    ids = np.asarray(flat_attr_ids).astype(np.int64)
    segs = np.asarray(attr_seg_ids).astype(np.int64)
    data = np.asarray(data).astype(np.int64)
    attr_table = np.ascontiguousarray(np.asarray(attr_table, dtype=np.float32))
    edge_emb = np.zeros((NSEG_PAD, D), np.float32)
    edge_emb[:NSEG] = np.asarray(edge_type_embedding, dtype=np.float32)
    edge_emb_zero = np.zeros((NSEG_PAD, D), np.float32)

    iotaf = np.tile(np.arange(128, dtype=np.float32)[None, :], (128, 1)).astype(bf16)
    iotap = np.arange(128, dtype=np.float16)[:, None]

    in_maps = []
    dev_idx = []
    for k in range(NCORES):
        # ---- stage 1: this core's attr references, bucketed by seg>>7 ----
        lo_id, hi_id = k * ATTR_PER_CORE, (k + 1) * ATTR_PER_CORE
        sel = (ids >= lo_id) & (ids < hi_id)
        ids_k = ids[sel] - lo_id
        segs_k = segs[sel]
        aid = np.zeros(S1_SLOTS, np.int64)
        aseg = np.full(S1_SLOTS, -1.0, np.float32)
        for b in range(S1_BUCKETS):
            m = (segs_k >> 7) == b
            nb = int(m.sum())
            assert nb <= S1_SLOTS_PER_BUCKET, f"s1 bucket overflow core {k} bucket {b}: {nb}"
            base = b * S1_SLOTS_PER_BUCKET
            aid[base:base + nb] = ids_k[m]
            aseg[base:base + nb] = (segs_k[m] - 128 * b).astype(np.float32)
        # per-bucket 16-wrap, concatenated along columns
        aidx16 = np.concatenate(
            [_wrap16(aid[b * S1_SLOTS_PER_BUCKET:(b + 1) * S1_SLOTS_PER_BUCKET])
             for b in range(S1_BUCKETS)], axis=1).astype(np.int16)
        aidx16 = np.tile(aidx16, (8, 1))                       # [128, 512]
        # slot (b, c, p) -> asegf[p, b*8+c]
        asegf_arr = np.ascontiguousarray(
            aseg.reshape(S1_BUCKETS * S1_KT_PER_BUCKET, 128).T.astype(bf16))

        # ---- stage 2: bucket edges by type>>7, stable, padded to L ----
        shard = data[k * E:(k + 1) * E]
        cid = shard >> 7
        dv = np.full(NCHUNK * L, -1.0, np.float16)
        didx = np.empty(E, np.int64)
        for c in range(NCHUNK):
            pos = np.nonzero(cid == c)[0]
            ncnt = pos.shape[0]
            assert ncnt <= L, f"edge chunk overflow core {k} chunk {c}: {ncnt}"
            dv[c * L:c * L + ncnt] = (shard[pos] - 128 * c).astype(np.float16)
            didx[pos] = c * L + np.arange(ncnt)
        dev_idx.append(didx)

        in_maps.append({
            "attr_shard": np.ascontiguousarray(attr_table[lo_id:hi_id]),
            "edge_emb": edge_emb if k == 0 else edge_emb_zero,
            "aidx": np.ascontiguousarray(aidx16),
            "asegf": asegf_arr,
            "iotaf": iotaf,
            "iotap": iotap,
            "dval": np.ascontiguousarray(dv[None, :]),
        })
    return in_maps, dev_idx


def run(inputs, trace=False, trace_cores=None):
    nc = _build_program()
    in_maps, dev_idx = _prep_in_maps(**inputs)
    kwargs = {}
    if trace:
        kwargs = dict(trace=True)
        if trace_cores is not None:
            kwargs["trace_cores"] = trace_cores
    res = run_bass_kernel_spmd(nc, in_maps, core_ids=list(range(NCORES)), **kwargs)
    outp = np.empty((N, D), np.float32)
    for k in range(NCORES):
        outp[k * E:(k + 1) * E] = res.results[k]["out_dev"][dev_idx[k]]
    return outp, res


def kernel(**inputs) -> np.ndarray:
    outp, _ = run(inputs, trace=False)
    return outp


# revision 7
# speedup vs baseline: 2.8778x; 1.5429x over previous
"""EdgeEmbedding forward on 8 Trainium2 NeuronCores.

Computation (see reference):
    type_attr_sum[t] = sum_{j: attr_seg_ids[j]==t} attr_table[flat_attr_ids[j]]
    combined[t]      = edge_type_embedding[t] + type_attr_sum[t]        # [1000, 256]
    out[i]           = combined[data[i]]                                # [1M, 256]

Distribution / algorithm:
  Stage 1 (segment sum): the 50K ragged attr references are sharded across
  the 8 cores by attr-table row range.  Within a core the references are
  bucketed by seg>>7 (8 buckets of 1024 padded slots); each bucket is
  gathered with gpsimd.dma_gather, cast bf16, and reduced with one-hot PE
  matmuls into that bucket's 128-seg PSUM tile.  Each 128-seg chunk is
  AllReduced separately (pipelined across chunks); edge_type_embedding is
  folded in on core 0 only (its input; zeros elsewhere), yielding the
  combined table chunk-by-chunk.

  Stage 2 (edge gather): edges are sharded across cores (125K each) and,
  on the host, stably bucketed by type>>7 into 8 chunks padded to 16384
  rows.  The device holds each combined chunk in SBUF as fp16 and emits
  every output row with a one-hot matmul: oh[t_local, e] =
  (t_local == dval[e]) built on DVE from a uint8 dval tile (host
  pre-replicated across partitions), then PSUM[e, :] = oh.T @ chi_chunk.
  ACT evacuates PSUM in 4-tile batches, 2MB HWDGE DMAs write HBM.  No HBM
  gather reads - the only bulk HBM traffic is the output write plus a
  1-byte-per-edge index stream.  The host inverse-permutes rows on
  unshard (order within a chunk is preserved, pads dropped).
"""
import os
import sys

sys.path.insert(0, "/opt/trn_rl_repo")

import numpy as np

import concourse.bass as bass
import concourse.bacc as bacc
import concourse.mybir as mybir
from concourse.tile import TileContext
from concourse.bass_utils import run_bass_kernel_spmd

# ---- problem constants (hardcoded per harness contract) ----
N = 1_000_000
D = 256
NSEG = 1000
NSEG_PAD = 1024
ATTR_NUM = 200_000
NCORES = 8
ATTR_PER_CORE = ATTR_NUM // NCORES      # 25_000 table rows per core
E = N // NCORES                         # 125_000 edges per core

NCHUNK = 8            # type chunks of 128
L = 16_384            # padded edge slots per chunk (6.5 sigma above 15625 mean)
SEG = 4096            # edges per one-hot build granule
SEGS_PER_CHUNK = L // SEG               # 4
TILES_PER_SEG = SEG // 128              # 32
GROUP = 32            # 128-edge tiles per output DMA (2 MB fp16)
GROUPS_PER_SEG = TILES_PER_SEG // GROUP  # 1

S1_BUCKETS = 8        # seg chunks of 128
S1_SLOTS_PER_BUCKET = 1024              # 8.7 sigma above 781 mean
S1_KT_PER_BUCKET = S1_SLOTS_PER_BUCKET // 128   # 8
S1_SLOTS = S1_BUCKETS * S1_SLOTS_PER_BUCKET     # 8192

_cached = {}


def _build_program():
    if "nc" in _cached:
        return _cached["nc"]
    nc = bacc.Bacc("TRN2", target_bir_lowering=False, debug=False, num_devices=NCORES)

    f32 = mybir.dt.float32
    bf16 = mybir.dt.bfloat16
    fp16 = mybir.dt.float16
    u8 = mybir.dt.uint8

    attr_shard = nc.dram_tensor("attr_shard", [ATTR_PER_CORE, D], f32, kind="ExternalInput")
    edge_emb = nc.dram_tensor("edge_emb", [NSEG_PAD, D], f32, kind="ExternalInput")
    aidx = nc.dram_tensor("aidx", [128, S1_SLOTS // 16], mybir.dt.int16, kind="ExternalInput")
    asegf = nc.dram_tensor("asegf", [128, S1_BUCKETS * S1_KT_PER_BUCKET], bf16, kind="ExternalInput")
    iotaf = nc.dram_tensor("iotaf", [128, 128], bf16, kind="ExternalInput")
    iotap = nc.dram_tensor("iotap", [128, 1], u8, kind="ExternalInput")
    dval = nc.dram_tensor("dval", [128, NCHUNK * L], u8, kind="ExternalInput")
    out_dev = nc.dram_tensor("out_dev", [NCHUNK * L, D], fp16, kind="ExternalOutput")

    ar_in = nc.dram_tensor("ar_in", [NSEG_PAD, D], f32)
    ar_out = nc.dram_tensor("ar_out", [NSEG_PAD, D], f32)

    with TileContext(nc) as tc:
        with (
            tc.tile_pool(name="misc", bufs=1) as misc,
            tc.tile_pool(name="s1a", bufs=2) as s1a,
            tc.tile_pool(name="s1oh", bufs=2) as s1oh,
            tc.tile_pool(name="s1ps", bufs=2, space="PSUM") as s1ps,
            tc.tile_pool(name="s1out", bufs=2) as s1out,
            tc.tile_pool(name="chip", bufs=2) as chip,
            tc.tile_pool(name="s2dvr", bufs=3) as s2dvr,
            tc.tile_pool(name="s2oh", bufs=3) as s2oh,
            tc.tile_pool(name="s2ps", bufs=3, space="PSUM") as s2ps,
            tc.tile_pool(name="s2st", bufs=3) as s2st,
        ):
            # ---- prologue: constants / index tables (SP ring) ----
            aidx_t = misc.tile([128, S1_SLOTS // 16], mybir.dt.int16)
            nc.sync.dma_start(out=aidx_t[:, :], in_=aidx.ap())
            asegf_t = misc.tile([128, S1_BUCKETS * S1_KT_PER_BUCKET], bf16)
            nc.sync.dma_start(out=asegf_t[:, :], in_=asegf.ap())
            iotaf_t = misc.tile([128, 128], bf16)
            nc.sync.dma_start(out=iotaf_t[:, :], in_=iotaf.ap())
            iotap_t = misc.tile([128, 1], u8)
            nc.sync.dma_start(out=iotap_t[:, :], in_=iotap.ap())

            for b in range(NCHUNK):
                # ================= stage 1, seg bucket b =================
                atile = s1a.tile([128, S1_KT_PER_BUCKET, D], f32, tag="atile", name=f"atile{b}")
                nc.gpsimd.dma_gather(
                    out_ap=atile[:, :, :],
                    in_ap=attr_shard.ap(),
                    idxs_ap=aidx_t[:, b * (S1_SLOTS_PER_BUCKET // 16):(b + 1) * (S1_SLOTS_PER_BUCKET // 16)],
                    num_idxs=S1_SLOTS_PER_BUCKET,
                    num_idxs_reg=S1_SLOTS_PER_BUCKET,
                    elem_size=D,
                    single_packet=False,
                )
                abf = s1a.tile([128, S1_KT_PER_BUCKET, D], bf16, tag="abf", name=f"abf{b}")
                nc.vector.tensor_copy(abf[:, :, :], atile[:, :, :])

                ps1 = s1ps.tile([128, D], f32, tag="ps1", name=f"ps1_{b}")
                for c in range(S1_KT_PER_BUCKET):
                    kt = b * S1_KT_PER_BUCKET + c
                    oh1 = s1oh.tile([128, 128], bf16, tag="oh1", name=f"oh1_{b}_{c}")
                    nc.vector.tensor_tensor(
                        oh1[:, :],
                        asegf_t[:, kt:kt + 1].broadcast_to((128, 128)),
                        iotaf_t[:, :],
                        op=mybir.AluOpType.is_equal,
                    )
                    nc.tensor.matmul(
                        ps1[:, :], oh1[:, :], abf[:, c, :],
                        start=(c == 0), stop=(c == S1_KT_PER_BUCKET - 1),
                    )
                part = s1out.tile([128, D], f32, tag="part", name=f"part{b}")
                nc.scalar.copy(part[:, :], ps1[:, :])
                embt = s1out.tile([128, D], f32, tag="embt", name=f"embt{b}")
                nc.scalar.dma_start(out=embt[:, :], in_=edge_emb.ap()[b * 128:(b + 1) * 128, :])
                nc.vector.tensor_add(part[:, :], part[:, :], embt[:, :])
                nc.sync.dma_start(out=ar_in.ap()[b * 128:(b + 1) * 128, :], in_=part[:, :])

                nc.gpsimd.collective_compute(
                    "AllReduce", mybir.AluOpType.add,
                    replica_groups=[list(range(NCORES))],
                    ins=[ar_in.ap()[b * 128:(b + 1) * 128, :].opt()],
                    outs=[ar_out.ap()[b * 128:(b + 1) * 128, :].opt()],
                )

                # ================= stage 2, type chunk b =================
                # one-hot prework (independent of the AllReduce)
                ohs = []
                for s in range(SEGS_PER_CHUNK):
                    off = b * L + s * SEG
                    dvr = s2dvr.tile([128, SEG], u8, tag="dvr", name=f"dvr{b}_{s}")
                    nc.scalar.dma_start(out=dvr[:, :], in_=dval.ap()[:, off:off + SEG])
                    oh2 = s2oh.tile([128, SEG], fp16, tag="oh2", name=f"oh2_{b}_{s}")
                    nc.vector.tensor_tensor(
                        oh2[:, :],
                        iotap_t[:, 0:1].broadcast_to((128, SEG)),
                        dvr[:, :],
                        op=mybir.AluOpType.is_equal,
                    )
                    ohs.append(oh2)

                # combined chunk: load f32, cast fp16 on ACT right before use
                ctmp = s1out.tile([128, D], f32, tag="ctmp", name=f"ctmp{b}")
                nc.scalar.dma_start(out=ctmp[:, :], in_=ar_out.ap()[b * 128:(b + 1) * 128, :])
                chi = chip.tile([128, D], fp16, tag="chi", name=f"chi{b}")
                nc.scalar.copy(chi[:, :], ctmp[:, :])

                for s in range(SEGS_PER_CHUNK):
                    off = b * L + s * SEG
                    oh2 = ohs[s]
                    for g in range(GROUPS_PER_SEG):
                        st = s2st.tile([128, GROUP, D], fp16, tag="st", name=f"st{b}_{s}_{g}")
                        for j in range(GROUP // 4):
                            pp = s2ps.tile([128, 4, D], f32, tag="pp", name=f"pp{b}_{s}_{g}_{j}")
                            for h in range(4):
                                u = g * GROUP + j * 4 + h
                                nc.tensor.matmul(
                                    pp[:, h, :],
                                    oh2[:, u * 128:(u + 1) * 128],
                                    chi[:, :],
                                    start=True, stop=True,
                                )
                            nc.scalar.copy(st[:, j * 4:j * 4 + 4, :], pp[:, :, :])
                        row0 = off + g * GROUP * 128
                        dst = bass.AP(out_dev, row0 * D, [[D, 128], [128 * D, GROUP], [1, D]])
                        nc.sync.dma_start(out=dst, in_=st[:, :, :])

    nc.compile()
    _cached["nc"] = nc
    return nc


def _wrap16(arr):
    """Position j -> [j%16, j//16] layout expected by dma_gather idx tensors."""
    assert arr.shape[0] % 16 == 0
    return arr.reshape(arr.shape[0] // 16, 16).T


def _prep_in_maps(data, attr_table, edge_type_embedding, flat_attr_ids, attr_seg_ids):
    import ml_dtypes
    bf16 = ml_dtypes.bfloat16

    ids = np.asarray(flat_attr_ids).astype(np.int64)
    segs = np.asarray(attr_seg_ids).astype(np.int64)
    data = np.asarray(data).astype(np.int64)
    attr_table = np.ascontiguousarray(np.asarray(attr_table, dtype=np.float32))
    edge_emb = np.zeros((NSEG_PAD, D), np.float32)
    edge_emb[:NSEG] = np.asarray(edge_type_embedding, dtype=np.float32)
    edge_emb_zero = np.zeros((NSEG_PAD, D), np.float32)

    iotaf = np.tile(np.arange(128, dtype=np.float32)[None, :], (128, 1)).astype(bf16)
    iotap = np.arange(128, dtype=np.uint8)[:, None]

    in_maps = []
    dev_idx = []
    for k in range(NCORES):
        # ---- stage 1: this core's attr references, bucketed by seg>>7 ----
        lo_id, hi_id = k * ATTR_PER_CORE, (k + 1) * ATTR_PER_CORE
        sel = (ids >= lo_id) & (ids < hi_id)
        ids_k = ids[sel] - lo_id
        segs_k = segs[sel]
        aid = np.zeros(S1_SLOTS, np.int64)
        aseg = np.full(S1_SLOTS, -1.0, np.float32)
        for b in range(S1_BUCKETS):
            m = (segs_k >> 7) == b
            nb = int(m.sum())
            assert nb <= S1_SLOTS_PER_BUCKET, f"s1 bucket overflow core {k} bucket {b}: {nb}"
            base = b * S1_SLOTS_PER_BUCKET
            aid[base:base + nb] = ids_k[m]
            aseg[base:base + nb] = (segs_k[m] - 128 * b).astype(np.float32)
        # per-bucket 16-wrap, concatenated along columns
        aidx16 = np.concatenate(
            [_wrap16(aid[b * S1_SLOTS_PER_BUCKET:(b + 1) * S1_SLOTS_PER_BUCKET])
             for b in range(S1_BUCKETS)], axis=1).astype(np.int16)
        aidx16 = np.tile(aidx16, (8, 1))                       # [128, 512]
        # slot (b, c, p) -> asegf[p, b*8+c]
        asegf_arr = np.ascontiguousarray(
            aseg.reshape(S1_BUCKETS * S1_KT_PER_BUCKET, 128).T.astype(bf16))

        # ---- stage 2: bucket edges by type>>7, stable, padded to L ----
        shard = data[k * E:(k + 1) * E]
        cid = shard >> 7
        dv = np.full(NCHUNK * L, 255, np.uint8)   # 255 = pad (never matches 0..127)
        didx = np.empty(E, np.int64)
        for c in range(NCHUNK):
            pos = np.nonzero(cid == c)[0]
            ncnt = pos.shape[0]
            assert ncnt <= L, f"edge chunk overflow core {k} chunk {c}: {ncnt}"
            dv[c * L:c * L + ncnt] = (shard[pos] - 128 * c).astype(np.uint8)
            didx[pos] = c * L + np.arange(ncnt)
        dev_idx.append(didx)

        in_maps.append({
            "attr_shard": np.ascontiguousarray(attr_table[lo_id:hi_id]),
            "edge_emb": edge_emb if k == 0 else edge_emb_zero,
            "aidx": np.ascontiguousarray(aidx16),
            "asegf": asegf_arr,
            "iotaf": iotaf,
            "iotap": iotap,
            "dval": np.ascontiguousarray(np.broadcast_to(dv[None, :], (128, NCHUNK * L))),
        })
    return in_maps, dev_idx


def run(inputs, trace=False, trace_cores=None):
    nc = _build_program()
    in_maps, dev_idx = _prep_in_maps(**inputs)
    kwargs = {}
    if trace:
        kwargs = dict(trace=True)
        if trace_cores is not None:
            kwargs["trace_cores"] = trace_cores
    res = run_bass_kernel_spmd(nc, in_maps, core_ids=list(range(NCORES)), **kwargs)
    outp = np.empty((N, D), np.float32)
    for k in range(NCORES):
        outp[k * E:(k + 1) * E] = res.results[k]["out_dev"][dev_idx[k]].astype(np.float32)
    return outp, res


def kernel(**inputs) -> np.ndarray:
    outp, _ = run(inputs, trace=False)
    return outp


# revision 10
# speedup vs baseline: 3.1634x; 1.0992x over previous
"""EdgeEmbedding forward on 8 Trainium2 NeuronCores.

Computation (see reference):
    type_attr_sum[t] = sum_{j: attr_seg_ids[j]==t} attr_table[flat_attr_ids[j]]
    combined[t]      = edge_type_embedding[t] + type_attr_sum[t]        # [1000, 256]
    out[i]           = combined[data[i]]                                # [1M, 256]

Distribution / algorithm:
  Stage 1 (segment sum): the 50K ragged attr references are sharded across
  the 8 cores by attr-table row range.  Within a core the references are
  bucketed by seg>>7 (8 buckets of 1024 padded slots); each bucket is
  gathered with gpsimd.dma_gather, cast bf16, and reduced with one-hot PE
  matmuls into that bucket's 128-seg PSUM tile.  Each 128-seg chunk is
  AllReduced separately (pipelined across chunks); edge_type_embedding is
  folded in on core 0 only (its input; zeros elsewhere), yielding the
  combined table chunk-by-chunk.

  Stage 2 (edge gather): edges are sharded across cores (125K each) and,
  on the host, stably bucketed by type>>7 into 8 chunks padded to 16384
  rows.  The device holds each combined chunk in SBUF as fp16 and emits
  every output row with a one-hot matmul: oh[t_local, e] =
  (t_local == dval[e]) built on DVE from a uint8 dval tile (host
  pre-replicated across partitions), then PSUM[e, :] = oh.T @ chi_chunk.
  ACT evacuates PSUM in 4-tile batches, 2MB HWDGE DMAs write HBM.  No HBM
  gather reads - the only bulk HBM traffic is the output write plus a
  1-byte-per-edge index stream.  The host inverse-permutes rows on
  unshard (order within a chunk is preserved, pads dropped).
"""
import os
import sys

sys.path.insert(0, "/opt/trn_rl_repo")

import numpy as np

import concourse.bass as bass
import concourse.bacc as bacc
import concourse.mybir as mybir
from concourse.tile import TileContext
from concourse.bass_utils import run_bass_kernel_spmd

# ---- problem constants (hardcoded per harness contract) ----
N = 1_000_000
D = 256
NSEG = 1000
NSEG_PAD = 1024
ATTR_NUM = 200_000
NCORES = 8
ATTR_PER_CORE = ATTR_NUM // NCORES      # 25_000 table rows per core
E = N // NCORES                         # 125_000 edges per core

NCHUNK = 8            # type chunks of 128
L = 16_384            # padded edge slots per chunk (6.5 sigma above 15625 mean)
SEG = 4096            # edges per one-hot build granule
SEGS_PER_CHUNK = L // SEG               # 4
TILES_PER_SEG = SEG // 128              # 32
GROUP = 32            # 128-edge tiles per output DMA (2 MB fp16)
GROUPS_PER_SEG = TILES_PER_SEG // GROUP  # 1

S1_BUCKETS = 8        # seg chunks of 128
S1_SLOTS_PER_BUCKET = 1024              # 8.7 sigma above 781 mean
S1_KT_PER_BUCKET = S1_SLOTS_PER_BUCKET // 128   # 8
S1_SLOTS = S1_BUCKETS * S1_SLOTS_PER_BUCKET     # 8192

_cached = {}


def _build_program():
    if "nc" in _cached:
        return _cached["nc"]
    nc = bacc.Bacc("TRN2", target_bir_lowering=False, debug=False, num_devices=NCORES)

    f32 = mybir.dt.float32
    bf16 = mybir.dt.bfloat16
    fp16 = mybir.dt.float16
    u8 = mybir.dt.uint8

    attr_shard = nc.dram_tensor("attr_shard", [ATTR_PER_CORE, D], f32, kind="ExternalInput")
    edge_emb = nc.dram_tensor("edge_emb", [NSEG_PAD, D], f32, kind="ExternalInput")
    aidx = nc.dram_tensor("aidx", [128, S1_SLOTS // 16], mybir.dt.int16, kind="ExternalInput")
    asegf = nc.dram_tensor("asegf", [128, S1_BUCKETS * S1_KT_PER_BUCKET], bf16, kind="ExternalInput")
    iotaf = nc.dram_tensor("iotaf", [128, 128], bf16, kind="ExternalInput")
    iotap = nc.dram_tensor("iotap", [128, 1], u8, kind="ExternalInput")
    dval = nc.dram_tensor("dval", [128, NCHUNK * L], u8, kind="ExternalInput")
    out_dev = nc.dram_tensor("out_dev", [NCHUNK * L, D], fp16, kind="ExternalOutput")

    ar_in = nc.dram_tensor("ar_in", [NSEG_PAD, D], f32)
    ar_out = nc.dram_tensor("ar_out", [NSEG_PAD, D], f32)

    with TileContext(nc) as tc:
        with (
            tc.tile_pool(name="misc", bufs=1) as misc,
            tc.tile_pool(name="s1a", bufs=2) as s1a,
            tc.tile_pool(name="s1oh", bufs=2) as s1oh,
            tc.tile_pool(name="s1ps", bufs=2, space="PSUM") as s1ps,
            tc.tile_pool(name="s1out", bufs=2) as s1out,
            tc.tile_pool(name="chip", bufs=NCHUNK) as chip,
            tc.tile_pool(name="s2dvr", bufs=3) as s2dvr,
            tc.tile_pool(name="s2oh", bufs=3) as s2oh,
            tc.tile_pool(name="s2ps", bufs=3, space="PSUM") as s2ps,
            tc.tile_pool(name="s2st", bufs=3) as s2st,
        ):
            # ---- prologue: constants / index tables (SP ring) ----
            aidx_t = misc.tile([128, S1_SLOTS // 16], mybir.dt.int16)
            nc.sync.dma_start(out=aidx_t[:, :], in_=aidx.ap())
            asegf_t = misc.tile([128, S1_BUCKETS * S1_KT_PER_BUCKET], bf16)
            nc.sync.dma_start(out=asegf_t[:, :], in_=asegf.ap())
            iotaf_t = misc.tile([128, 128], bf16)
            nc.sync.dma_start(out=iotaf_t[:, :], in_=iotaf.ap())
            iotap_t = misc.tile([128, 1], u8)
            nc.sync.dma_start(out=iotap_t[:, :], in_=iotap.ap())

            # ======== stage 1 for ALL buckets first: emission order is the
            # scheduler's priority, so the gather -> segment-sum -> AllReduce
            # -> chi ladder always wins the per-engine ready-heap and stage-2
            # work below fills the idle slots around it. ========
            chis = []
            for b in range(NCHUNK):
                # ================= stage 1, seg bucket b =================
                atile = s1a.tile([128, S1_KT_PER_BUCKET, D], f32, tag="atile", name=f"atile{b}")
                nc.gpsimd.dma_gather(
                    out_ap=atile[:, :, :],
                    in_ap=attr_shard.ap(),
                    idxs_ap=aidx_t[:, b * (S1_SLOTS_PER_BUCKET // 16):(b + 1) * (S1_SLOTS_PER_BUCKET // 16)],
                    num_idxs=S1_SLOTS_PER_BUCKET,
                    num_idxs_reg=S1_SLOTS_PER_BUCKET,
                    elem_size=D,
                    single_packet=False,
                )
                abf = s1a.tile([128, S1_KT_PER_BUCKET, D], bf16, tag="abf", name=f"abf{b}")
                nc.vector.tensor_copy(abf[:, :, :], atile[:, :, :])

                ps1 = s1ps.tile([128, D], f32, tag="ps1", name=f"ps1_{b}")
                for c in range(S1_KT_PER_BUCKET):
                    kt = b * S1_KT_PER_BUCKET + c
                    oh1 = s1oh.tile([128, 128], bf16, tag="oh1", name=f"oh1_{b}_{c}")
                    nc.vector.tensor_tensor(
                        oh1[:, :],
                        asegf_t[:, kt:kt + 1].broadcast_to((128, 128)),
                        iotaf_t[:, :],
                        op=mybir.AluOpType.is_equal,
                    )
                    nc.tensor.matmul(
                        ps1[:, :], oh1[:, :], abf[:, c, :],
                        start=(c == 0), stop=(c == S1_KT_PER_BUCKET - 1),
                    )
                part = s1out.tile([128, D], f32, tag="part", name=f"part{b}")
                nc.scalar.copy(part[:, :], ps1[:, :])
                embt = s1out.tile([128, D], f32, tag="embt", name=f"embt{b}")
                nc.scalar.dma_start(out=embt[:, :], in_=edge_emb.ap()[b * 128:(b + 1) * 128, :])
                nc.vector.tensor_add(part[:, :], part[:, :], embt[:, :])
                nc.sync.dma_start(out=ar_in.ap()[b * 128:(b + 1) * 128, :], in_=part[:, :])

                nc.gpsimd.collective_compute(
                    "AllReduce", mybir.AluOpType.add,
                    replica_groups=[list(range(NCORES))],
                    ins=[ar_in.ap()[b * 128:(b + 1) * 128, :].opt()],
                    outs=[ar_out.ap()[b * 128:(b + 1) * 128, :].opt()],
                )

                # combined chunk: load f32, cast fp16 on ACT right before use
                ctmp = s1out.tile([128, D], f32, tag="ctmp", name=f"ctmp{b}")
                nc.scalar.dma_start(out=ctmp[:, :], in_=ar_out.ap()[b * 128:(b + 1) * 128, :])
                chi = chip.tile([128, D], fp16, tag="chi", name=f"chi{b}")
                nc.scalar.copy(chi[:, :], ctmp[:, :])
                chis.append(chi)

            # ======== stage 2: all chunks ========
            for b in range(NCHUNK):
                chi = chis[b]
                for s in range(SEGS_PER_CHUNK):
                    off = b * L + s * SEG
                    dvr = s2dvr.tile([128, SEG], u8, tag="dvr", name=f"dvr{b}_{s}")
                    nc.scalar.dma_start(out=dvr[:, :], in_=dval.ap()[:, off:off + SEG])
                    oh2 = s2oh.tile([128, SEG], fp16, tag="oh2", name=f"oh2_{b}_{s}")
                    nc.vector.tensor_tensor(
                        oh2[:, :],
                        iotap_t[:, 0:1].broadcast_to((128, SEG)),
                        dvr[:, :],
                        op=mybir.AluOpType.is_equal,
                    )
                    for g in range(GROUPS_PER_SEG):
                        st = s2st.tile([128, GROUP, D], fp16, tag="st", name=f"st{b}_{s}_{g}")
                        for j in range(GROUP // 4):
                            pp = s2ps.tile([128, 4, D], f32, tag="pp", name=f"pp{b}_{s}_{g}_{j}")
                            for h in range(4):
                                u = g * GROUP + j * 4 + h
                                nc.tensor.matmul(
                                    pp[:, h, :],
                                    oh2[:, u * 128:(u + 1) * 128],
                                    chi[:, :],
                                    start=True, stop=True,
                                )
                            if j % 4 == 3:
                                nc.vector.tensor_copy(st[:, j * 4:j * 4 + 4, :], pp[:, :, :])
                            else:
                                nc.scalar.copy(st[:, j * 4:j * 4 + 4, :], pp[:, :, :])
                        row0 = off + g * GROUP * 128
                        dst = bass.AP(out_dev, row0 * D, [[D, 128], [128 * D, GROUP], [1, D]])
                        nc.sync.dma_start(out=dst, in_=st[:, :, :])

    nc.compile()
    _cached["nc"] = nc
    return nc


def _wrap16(arr):
    """Position j -> [j%16, j//16] layout expected by dma_gather idx tensors."""
    assert arr.shape[0] % 16 == 0
    return arr.reshape(arr.shape[0] // 16, 16).T


def _prep_in_maps(data, attr_table, edge_type_embedding, flat_attr_ids, attr_seg_ids):
    import ml_dtypes
    bf16 = ml_dtypes.bfloat16

    ids = np.asarray(flat_attr_ids).astype(np.int64)
    segs = np.asarray(attr_seg_ids).astype(np.int64)
    data = np.asarray(data).astype(np.int64)
    attr_table = np.ascontiguousarray(np.asarray(attr_table, dtype=np.float32))
    edge_emb = np.zeros((NSEG_PAD, D), np.float32)
    edge_emb[:NSEG] = np.asarray(edge_type_embedding, dtype=np.float32)
    edge_emb_zero = np.zeros((NSEG_PAD, D), np.float32)

    iotaf = np.tile(np.arange(128, dtype=np.float32)[None, :], (128, 1)).astype(bf16)
    iotap = np.arange(128, dtype=np.uint8)[:, None]

    in_maps = []
    dev_idx = []
    for k in range(NCORES):
        # ---- stage 1: this core's attr references, bucketed by seg>>7 ----
        lo_id, hi_id = k * ATTR_PER_CORE, (k + 1) * ATTR_PER_CORE
        sel = (ids >= lo_id) & (ids < hi_id)
        ids_k = ids[sel] - lo_id
        segs_k = segs[sel]
        aid = np.zeros(S1_SLOTS, np.int64)
        aseg = np.full(S1_SLOTS, -1.0, np.float32)
        for b in range(S1_BUCKETS):
            m = (segs_k >> 7) == b
            nb = int(m.sum())
            assert nb <= S1_SLOTS_PER_BUCKET, f"s1 bucket overflow core {k} bucket {b}: {nb}"
            base = b * S1_SLOTS_PER_BUCKET
            aid[base:base + nb] = ids_k[m]
            aseg[base:base + nb] = (segs_k[m] - 128 * b).astype(np.float32)
        # per-bucket 16-wrap, concatenated along columns
        aidx16 = np.concatenate(
            [_wrap16(aid[b * S1_SLOTS_PER_BUCKET:(b + 1) * S1_SLOTS_PER_BUCKET])
             for b in range(S1_BUCKETS)], axis=1).astype(np.int16)
        aidx16 = np.tile(aidx16, (8, 1))                       # [128, 512]
        # slot (b, c, p) -> asegf[p, b*8+c]
        asegf_arr = np.ascontiguousarray(
            aseg.reshape(S1_BUCKETS * S1_KT_PER_BUCKET, 128).T.astype(bf16))

        # ---- stage 2: bucket edges by type>>7, stable, padded to L ----
        shard = data[k * E:(k + 1) * E]
        cid = shard >> 7
        dv = np.full(NCHUNK * L, 255, np.uint8)   # 255 = pad (never matches 0..127)
        didx = np.empty(E, np.int64)
        for c in range(NCHUNK):
            pos = np.nonzero(cid == c)[0]
            ncnt = pos.shape[0]
            assert ncnt <= L, f"edge chunk overflow core {k} chunk {c}: {ncnt}"
            dv[c * L:c * L + ncnt] = (shard[pos] - 128 * c).astype(np.uint8)
            didx[pos] = c * L + np.arange(ncnt)
        dev_idx.append(didx)

        in_maps.append({
            "attr_shard": np.ascontiguousarray(attr_table[lo_id:hi_id]),
            "edge_emb": edge_emb if k == 0 else edge_emb_zero,
            "aidx": np.ascontiguousarray(aidx16),
            "asegf": asegf_arr,
            "iotaf": iotaf,
            "iotap": iotap,
            "dval": np.ascontiguousarray(np.broadcast_to(dv[None, :], (128, NCHUNK * L))),
        })
    return in_maps, dev_idx


def run(inputs, trace=False, trace_cores=None):
    nc = _build_program()
    in_maps, dev_idx = _prep_in_maps(**inputs)
    kwargs = {}
    if trace:
        kwargs = dict(trace=True)
        if trace_cores is not None:
            kwargs["trace_cores"] = trace_cores
    res = run_bass_kernel_spmd(nc, in_maps, core_ids=list(range(NCORES)), **kwargs)
    outp = np.empty((N, D), np.float32)
    for k in range(NCORES):
        outp[k * E:(k + 1) * E] = res.results[k]["out_dev"][dev_idx[k]].astype(np.float32)
    return outp, res


def kernel(**inputs) -> np.ndarray:
    outp, _ = run(inputs, trace=False)
    return outp
